# revision 3
# baseline (speedup 1.0000x reference)
"""Dual-score attention kernel for Trainium2 (8 NeuronCores).

Computes, for inputs q_val/q_pos [B,L,H,E], k_val/k_pos/v_val/v_pos [B,S,H,E]:
    scores = einsum('blhe,bshe->bhls', q_val, k_val)
           + einsum('blhe,bshe->bhls', q_pos, k_pos)
    A  = softmax(scores / sqrt(E), axis=-1)
    V  = einsum('bhls,bshe->blhe', A, v_val)
    Vp = einsum('bhls,bshe->blhe', A, v_pos)
    returns (V, Vp, None)          # attn_mask is (faithfully) ignored

Sharding: the 16 (b,h) pairs are independent; each of the 8 cores handles 2.

Device-side layout trick: val/pos are concatenated along E (64+64=128) so the
dual-score sum is a single K=128 matmul. Scores are computed transposed
(St[s,l]) so that the softmax reduction over s lands on the PE partition dim,
where it is computed by a ones-vector matmul, and so that P=exp(St) is already
in the right layout to be the moving operand of the A@V matmul (stationary
Vcat = [v_val | v_pos], output [e'=128, l] accumulated over s tiles in PSUM).
"""

import os
import sys

import numpy as np

for _p in ("/opt/trn_rl_repo", "/root/.axon_site/_ro/trn_rl_repo"):
    if os.path.isdir(_p) and _p not in sys.path:
        sys.path.append(_p)

import ml_dtypes

import concourse.bass as bass
import concourse.tile as tile
from concourse import bacc, mybir
from concourse.bass_utils import run_bass_kernel_spmd

B, L, S, H, E = 2, 2048, 2048, 8, 64
NCORES = 8
NPAIR = 2          # (b,h) pairs per core
NT = S // 128      # 16 s-tiles
LH = 2             # l halves (PSUM budget)
LHW = L // LH      # 1024
NB = 512           # matmul free-dim chunk (one PSUM bank fp32)
SCALE = 1.0 / float(np.sqrt(E))

BF16 = mybir.dt.bfloat16
F32 = mybir.dt.float32

_CACHE = {}


def _build():
    nc = bacc.Bacc("TRN2", target_bir_lowering=False)

    qT = nc.dram_tensor("qT", [NPAIR, 128, L], BF16, kind="ExternalInput")
    kT = nc.dram_tensor("kT", [NPAIR, 128, S], BF16, kind="ExternalInput")
    vc = nc.dram_tensor("vc", [NPAIR, 128, NT, 128], BF16, kind="ExternalInput")
    out = nc.dram_tensor("out", [NPAIR, 128, L], F32, kind="ExternalOutput")

    with tile.TileContext(nc) as tc:
        with (
            tc.tile_pool(name="consts", bufs=1) as consts,
            tc.tile_pool(name="qk", bufs=2) as qk,
            tc.tile_pool(name="vpool", bufs=2) as vpool,
            tc.tile_pool(name="ppool", bufs=3) as ppool,
            tc.tile_pool(name="tailpool", bufs=2) as tailpool,
            tc.tile_pool(name="opool", bufs=2) as opool,
            tc.tile_pool(name="st_ps", bufs=2, space="PSUM") as st_ps,
            tc.tile_pool(name="pv_ps", bufs=4, space="PSUM") as pv_ps,
            tc.tile_pool(name="rs_ps", bufs=2, space="PSUM") as rs_ps,
        ):
            ones = consts.tile([128, 1], BF16, tag="ones")
            nc.vector.memset(ones[:], 1.0)

            for pr in range(NPAIR):
                qt = qk.tile([128, L], BF16, tag="qt")
                kt = qk.tile([128, S], BF16, tag="kt")
                vt = vpool.tile([128, NT, 128], BF16, tag="vt")
                nc.sync.dma_start(out=qt[:], in_=qT[pr])
                nc.sync.dma_start(out=kt[:], in_=kT[pr])
                nc.sync.dma_start(out=vt[:], in_=vc[pr])

                for lh in range(LH):
                    pv = [
                        pv_ps.tile([128, NB], F32, tag="pv", name=f"pv{pr}{lh}{c}")
                        for c in range(2)
                    ]
                    rs = [
                        rs_ps.tile([1, NB], F32, tag="rs", name=f"rs{pr}{lh}{c}")
                        for c in range(2)
                    ]

                    for i in range(NT):
                        p_i = ppool.tile([128, LHW], BF16, tag="p")
                        for c in range(2):
                            st = st_ps.tile([128, NB], F32, tag="st")
                            nc.tensor.matmul(
                                st[:],
                                kt[:, i * 128:(i + 1) * 128],
                                qt[:, lh * LHW + c * NB: lh * LHW + (c + 1) * NB],
                                start=True,
                                stop=True,
                            )
                            nc.scalar.activation(
                                p_i[:, c * NB:(c + 1) * NB],
                                st[:],
                                mybir.ActivationFunctionType.Exp,
                                scale=SCALE,
                            )
                        for c in range(2):
                            nc.tensor.matmul(
                                pv[c][:],
                                vt[:, i, :],
                                p_i[:, c * NB:(c + 1) * NB],
                                start=(i == 0),
                                stop=(i == NT - 1),
                            )
                        for c in range(2):
                            nc.tensor.matmul(
                                rs[c][:],
                                ones[:],
                                p_i[:, c * NB:(c + 1) * NB],
                                start=(i == 0),
                                stop=(i == NT - 1),
                            )

                    # tail: out[:, lh] = pv / rowsum  (broadcast along e')
                    recip = tailpool.tile([1, LHW], F32, tag="recip")
                    for c in range(2):
                        nc.vector.reciprocal(
                            recip[:, c * NB:(c + 1) * NB], rs[c][:]
                        )
                    rrep = tailpool.tile([128, LHW], F32, tag="rrep")
                    nc.gpsimd.partition_broadcast(rrep[:], recip[:])
                    o_sb = opool.tile([128, LHW], F32, tag="o")
                    for c in range(2):
                        nc.vector.tensor_mul(
                            o_sb[:, c * NB:(c + 1) * NB],
                            pv[c][:],
                            rrep[:, c * NB:(c + 1) * NB],
                        )
                    nc.sync.dma_start(
                        out=out[pr, :, lh * LHW:(lh + 1) * LHW], in_=o_sb[:]
                    )

    nc.compile()
    return nc


def _get_nc():
    if "nc" not in _CACHE:
        _CACHE["nc"] = _build()
    return _CACHE["nc"]


def _prep_inputs(q_val, q_pos, k_val, k_pos, v_val, v_pos):
    bf16 = ml_dtypes.bfloat16
    # [B,L,H,2E] -> [B,H,2E,L] -> [16, 128, L]
    qcat = np.concatenate([q_val, q_pos], axis=-1).transpose(0, 2, 3, 1)
    qcatT = np.ascontiguousarray(qcat.reshape(B * H, 2 * E, L)).astype(bf16)
    kcat = np.concatenate([k_val, k_pos], axis=-1).transpose(0, 2, 3, 1)
    kcatT = np.ascontiguousarray(kcat.reshape(B * H, 2 * E, S)).astype(bf16)
    # [B,S,H,2E] -> [B,H,S,2E] -> [16, NT, 128, 128] -> [16, 128(s), NT, 128(e)]
    vcat = np.concatenate([v_val, v_pos], axis=-1).transpose(0, 2, 1, 3)
    vtiles = vcat.reshape(B * H, NT, 128, 2 * E).transpose(0, 2, 1, 3)
    vtiles = np.ascontiguousarray(vtiles).astype(bf16)
    return [
        {
            "qT": np.ascontiguousarray(qcatT[2 * c: 2 * c + 2]),
            "kT": np.ascontiguousarray(kcatT[2 * c: 2 * c + 2]),
            "vc": np.ascontiguousarray(vtiles[2 * c: 2 * c + 2]),
        }
        for c in range(NCORES)
    ]


def kernel(q_val, q_pos, k_val, k_pos, v_val, v_pos, attn_mask=None):
    q_val, q_pos, k_val, k_pos, v_val, v_pos = (
        np.asarray(x, dtype=np.float32)
        for x in (q_val, q_pos, k_val, k_pos, v_val, v_pos)
    )
    nc = _get_nc()
    in_maps = _prep_inputs(q_val, q_pos, k_val, k_pos, v_val, v_pos)
    res = run_bass_kernel_spmd(nc, in_maps, core_ids=list(range(NCORES)))
    _CACHE["last_results"] = res
    outs = np.stack([np.asarray(r["out"], dtype=np.float32) for r in res.results])
    # [8, NPAIR, 128, L] -> [16, 128, L] -> [B, H, L, 128]
    o = outs.reshape(B * H, 2 * E, L).transpose(0, 2, 1).reshape(B, H, L, 2 * E)
    V = np.ascontiguousarray(o[..., :E].transpose(0, 2, 1, 3), dtype=np.float32)
    Vp = np.ascontiguousarray(o[..., E:].transpose(0, 2, 1, 3), dtype=np.float32)
    return (V, Vp, None)


# revision 5
# speedup vs baseline: 1.1059x; 1.1059x over previous
"""Dual-score attention kernel for Trainium2 (8 NeuronCores).

Computes, for inputs q_val/q_pos [B,L,H,E], k_val/k_pos/v_val/v_pos [B,S,H,E]:
    scores = einsum('blhe,bshe->bhls', q_val, k_val)
           + einsum('blhe,bshe->bhls', q_pos, k_pos)
    A  = softmax(scores / sqrt(E), axis=-1)
    V  = einsum('bhls,bshe->blhe', A, v_val)
    Vp = einsum('bhls,bshe->blhe', A, v_pos)
    returns (V, Vp, None)          # attn_mask is (faithfully) ignored

Sharding: the 16 (b,h) pairs are independent; each of the 8 cores handles 2.

Device-side layout trick: val/pos are concatenated along E (64+64=128) so the
dual-score sum is a single K=128 matmul. Scores are computed transposed
(St[s,l]) so that the softmax reduction over s lands on the PE partition dim,
where it is computed by a ones-vector matmul, and so that P=exp(St) is already
in the right layout to be the moving operand of the A@V matmul (stationary
Vcat = [v_val | v_pos], output [e'=128, l] accumulated over s tiles in PSUM).
"""

import os
import sys

import numpy as np

for _p in ("/opt/trn_rl_repo", "/root/.axon_site/_ro/trn_rl_repo"):
    if os.path.isdir(_p) and _p not in sys.path:
        sys.path.append(_p)

import ml_dtypes

import concourse.bass as bass
import concourse.tile as tile
from concourse import bacc, mybir
from concourse.bass_utils import run_bass_kernel_spmd

B, L, S, H, E = 2, 2048, 2048, 8, 64
NCORES = 8
NPAIR = 2          # (b,h) pairs per core
NT = S // 128      # 16 s-tiles
LH = 2             # l halves (PSUM budget)
LHW = L // LH      # 1024
NB = 512           # matmul free-dim chunk (one PSUM bank fp32)
SCALE = 1.0 / float(np.sqrt(E))

BF16 = mybir.dt.bfloat16
F32 = mybir.dt.float32

_CACHE = {}


def _build():
    nc = bacc.Bacc("TRN2", target_bir_lowering=False)

    qT = nc.dram_tensor("qT", [NPAIR, 128, L], BF16, kind="ExternalInput")
    kT = nc.dram_tensor("kT", [NPAIR, 128, S], BF16, kind="ExternalInput")
    vc = nc.dram_tensor("vc", [NPAIR, 128, NT, 128], BF16, kind="ExternalInput")
    out = nc.dram_tensor("out", [NPAIR, 128, L], F32, kind="ExternalOutput")

    with tile.TileContext(nc) as tc:
        with (
            tc.tile_pool(name="consts", bufs=1) as consts,
            tc.tile_pool(name="qk", bufs=2) as qk,
            tc.tile_pool(name="vpool", bufs=2) as vpool,
            tc.tile_pool(name="ppool", bufs=3) as ppool,
            tc.tile_pool(name="tailpool", bufs=2) as tailpool,
            tc.tile_pool(name="opool", bufs=2) as opool,
            tc.tile_pool(name="st_ps", bufs=2, space="PSUM") as st_ps,
            tc.tile_pool(name="pv_ps", bufs=2, space="PSUM") as pv_ps,
            tc.tile_pool(name="rs_ps", bufs=2, space="PSUM") as rs_ps,
        ):
            ones = consts.tile([128, 1], BF16, tag="ones")
            nc.vector.memset(ones[:], 1.0)

            for pr in range(NPAIR):
                qt = qk.tile([128, L], BF16, tag="qt")
                kt = qk.tile([128, S], BF16, tag="kt")
                vt = vpool.tile([128, NT, 128], BF16, tag="vt")
                # chunked loads so the first matmuls can start early
                for c in range(4):
                    nc.sync.dma_start(
                        out=kt[:, c * 512:(c + 1) * 512],
                        in_=kT[pr, :, c * 512:(c + 1) * 512],
                    )
                    if c < 2:
                        nc.sync.dma_start(
                            out=qt[:, c * LHW:(c + 1) * LHW],
                            in_=qT[pr, :, c * LHW:(c + 1) * LHW],
                        )
                nc.sync.dma_start(out=vt[:], in_=vc[pr])

                for lh in range(LH):
                    pv = [
                        pv_ps.tile([128, NB], F32, tag="pv", name=f"pv{pr}{lh}{c}")
                        for c in range(2)
                    ]
                    rs = [
                        rs_ps.tile([1, NB], F32, tag="rs", name=f"rs{pr}{lh}{c}")
                        for c in range(2)
                    ]

                    for i in range(NT):
                        p_i = ppool.tile([128, LHW], BF16, tag="p")
                        st = st_ps.tile([128, LHW], F32, tag="st")
                        for c in range(2):
                            nc.tensor.matmul(
                                st[:, c * NB:(c + 1) * NB],
                                kt[:, i * 128:(i + 1) * 128],
                                qt[:, lh * LHW + c * NB: lh * LHW + (c + 1) * NB],
                                start=True,
                                stop=True,
                            )
                        nc.scalar.activation(
                            p_i[:],
                            st[:],
                            mybir.ActivationFunctionType.Exp,
                            scale=SCALE,
                        )
                        for c in range(2):
                            nc.tensor.matmul(
                                pv[c][:],
                                vt[:, i, :],
                                p_i[:, c * NB:(c + 1) * NB],
                                start=(i == 0),
                                stop=(i == NT - 1),
                            )
                        for c in range(2):
                            nc.tensor.matmul(
                                rs[c][:],
                                ones[:],
                                p_i[:, c * NB:(c + 1) * NB],
                                start=(i == 0),
                                stop=(i == NT - 1),
                            )

                    # tail: out[:, lh] = pv / rowsum  (broadcast along e')
                    recip = tailpool.tile([1, LHW], F32, tag="recip")
                    for c in range(2):
                        nc.vector.reciprocal_approx_fast(
                            out=recip[:, c * NB:(c + 1) * NB], in_=rs[c][:]
                        )
                    rrep = tailpool.tile([128, LHW], F32, tag="rrep")
                    nc.gpsimd.partition_broadcast(rrep[:], recip[:])
                    o_sb = opool.tile([128, LHW], F32, tag="o")
                    for c in range(2):
                        nc.vector.tensor_mul(
                            o_sb[:, c * NB:(c + 1) * NB],
                            pv[c][:],
                            rrep[:, c * NB:(c + 1) * NB],
                        )
                        nc.sync.dma_start(
                            out=out[pr, :, lh * LHW + c * NB: lh * LHW + (c + 1) * NB],
                            in_=o_sb[:, c * NB:(c + 1) * NB],
                        )

    nc.compile()
    return nc


def _get_nc():
    if "nc" not in _CACHE:
        _CACHE["nc"] = _build()
    return _CACHE["nc"]


def _prep_inputs(q_val, q_pos, k_val, k_pos, v_val, v_pos):
    bf16 = ml_dtypes.bfloat16
    # [B,L,H,2E] -> [B,H,2E,L] -> [16, 128, L]
    qcat = np.concatenate([q_val, q_pos], axis=-1).transpose(0, 2, 3, 1)
    qcatT = np.ascontiguousarray(qcat.reshape(B * H, 2 * E, L)).astype(bf16)
    kcat = np.concatenate([k_val, k_pos], axis=-1).transpose(0, 2, 3, 1)
    kcatT = np.ascontiguousarray(kcat.reshape(B * H, 2 * E, S)).astype(bf16)
    # [B,S,H,2E] -> [B,H,S,2E] -> [16, NT, 128, 128] -> [16, 128(s), NT, 128(e)]
    vcat = np.concatenate([v_val, v_pos], axis=-1).transpose(0, 2, 1, 3)
    vtiles = vcat.reshape(B * H, NT, 128, 2 * E).transpose(0, 2, 1, 3)
    vtiles = np.ascontiguousarray(vtiles).astype(bf16)
    return [
        {
            "qT": np.ascontiguousarray(qcatT[2 * c: 2 * c + 2]),
            "kT": np.ascontiguousarray(kcatT[2 * c: 2 * c + 2]),
            "vc": np.ascontiguousarray(vtiles[2 * c: 2 * c + 2]),
        }
        for c in range(NCORES)
    ]


def kernel(q_val, q_pos, k_val, k_pos, v_val, v_pos, attn_mask=None):
    q_val, q_pos, k_val, k_pos, v_val, v_pos = (
        np.asarray(x, dtype=np.float32)
        for x in (q_val, q_pos, k_val, k_pos, v_val, v_pos)
    )
    nc = _get_nc()
    in_maps = _prep_inputs(q_val, q_pos, k_val, k_pos, v_val, v_pos)
    res = run_bass_kernel_spmd(nc, in_maps, core_ids=list(range(NCORES)))
    _CACHE["last_results"] = res
    outs = np.stack([np.asarray(r["out"], dtype=np.float32) for r in res.results])
    # [8, NPAIR, 128, L] -> [16, 128, L] -> [B, H, L, 128]
    o = outs.reshape(B * H, 2 * E, L).transpose(0, 2, 1).reshape(B, H, L, 2 * E)
    V = np.ascontiguousarray(o[..., :E].transpose(0, 2, 1, 3), dtype=np.float32)
    Vp = np.ascontiguousarray(o[..., E:].transpose(0, 2, 1, 3), dtype=np.float32)
    return (V, Vp, None)


# revision 8
# speedup vs baseline: 1.3909x; 1.2576x over previous
"""Dual-score attention kernel for Trainium2 (8 NeuronCores).

Computes, for inputs q_val/q_pos [B,L,H,E], k_val/k_pos/v_val/v_pos [B,S,H,E]:
    scores = einsum('blhe,bshe->bhls', q_val, k_val)
           + einsum('blhe,bshe->bhls', q_pos, k_pos)
    A  = softmax(scores / sqrt(E), axis=-1)
    V  = einsum('bhls,bshe->blhe', A, v_val)
    Vp = einsum('bhls,bshe->blhe', A, v_pos)
    returns (V, Vp, None)          # attn_mask is (faithfully) ignored

Sharding: the 16 (b,h) pairs are independent; each of the 8 cores handles 2.

Device-side layout trick: val/pos are concatenated along E (64+64=128) so the
dual-score sum is a single K=128 matmul. Scores are computed transposed
(St[s,l]) so that the softmax reduction over s lands on the PE partition dim,
where it is computed by a ones-vector matmul, and so that P=exp(St) is already
in the right layout to be the moving operand of the A@V matmul (stationary
Vcat = [v_val | v_pos], output [e'=128, l] accumulated over s tiles in PSUM).
"""

import os
import sys

import numpy as np

for _p in ("/opt/trn_rl_repo", "/root/.axon_site/_ro/trn_rl_repo"):
    if os.path.isdir(_p) and _p not in sys.path:
        sys.path.append(_p)

import ml_dtypes

import concourse.bass as bass
import concourse.tile as tile
from concourse import bacc, mybir
from concourse.bass_utils import run_bass_kernel_spmd

B, L, S, H, E = 2, 2048, 2048, 8, 64
NCORES = 8
NPAIR = 2          # (b,h) pairs per core
NT = S // 128      # 16 s-tiles
LH = 2             # l halves (PSUM budget)
LHW = L // LH      # 1024
NB = 512           # matmul free-dim chunk (one PSUM bank fp32)
SCALE = 1.0 / float(np.sqrt(E))

BF16 = mybir.dt.bfloat16
F32 = mybir.dt.float32

_CACHE = {}


def _build():
    nc = bacc.Bacc("TRN2", target_bir_lowering=False)

    qT = nc.dram_tensor("qT", [NPAIR, 128, L], BF16, kind="ExternalInput")
    kT = nc.dram_tensor("kT", [NPAIR, 128, S], BF16, kind="ExternalInput")
    vc = nc.dram_tensor("vc", [NPAIR, 128, NT, 128], BF16, kind="ExternalInput")
    out = nc.dram_tensor("out", [NPAIR, 128, L], F32, kind="ExternalOutput")

    with tile.TileContext(nc) as tc:
        with (
            tc.tile_pool(name="consts", bufs=1) as consts,
            tc.tile_pool(name="qk", bufs=2) as qk,
            tc.tile_pool(name="vpool", bufs=2) as vpool,
            tc.tile_pool(name="ppool", bufs=18) as ppool,
            tc.tile_pool(name="tailpool", bufs=2) as tailpool,
            tc.tile_pool(name="opool", bufs=2) as opool,
            tc.tile_pool(name="st_ps", bufs=2, space="PSUM") as st_ps,
            tc.tile_pool(name="pv_ps", bufs=2, space="PSUM") as pv_ps,
            tc.tile_pool(name="rs_ps", bufs=2, space="PSUM") as rs_ps,
        ):
            ones = consts.tile([128, 1], BF16, tag="ones")
            nc.vector.memset(ones[:], 1.0)
            # HAM warmup: junk matmuls keep the PE busy during the initial
            # DMA wait so real matmuls start at 2.4 GHz instead of 1.2.
            wz = consts.tile([128, 128], BF16, tag="wz")
            nc.vector.memset(wz[:], 0.0)
            for w in range(28):
                warm = rs_ps.tile([1, 128], F32, tag="rs", name=f"warm{w}")
                nc.tensor.matmul(warm[:], ones[:], wz[:], start=True, stop=True)

            for pr in range(NPAIR):
                qt = qk.tile([128, L], BF16, tag="qt")
                kt = qk.tile([128, S], BF16, tag="kt")
                vt = vpool.tile([128, NT, 128], BF16, tag="vt")
                # chunked loads spread over distinct engine queues so they
                # land in parallel and the first matmuls start early
                for c in range(4):
                    nc.sync.dma_start(
                        out=kt[:, c * 512:(c + 1) * 512],
                        in_=kT[pr, :, c * 512:(c + 1) * 512],
                    )
                for c in range(2):
                    nc.scalar.dma_start(
                        out=qt[:, c * LHW:(c + 1) * LHW],
                        in_=qT[pr, :, c * LHW:(c + 1) * LHW],
                    )
                for c in range(2):
                    nc.gpsimd.dma_start(
                        out=vt[:, 8 * c:8 * (c + 1), :],
                        in_=vc[pr, :, 8 * c:8 * (c + 1), :],
                    )

                for lh in range(LH):
                    pv = [
                        pv_ps.tile([128, NB], F32, tag="pv", name=f"pv{pr}{lh}{c}")
                        for c in range(2)
                    ]
                    rs = [
                        rs_ps.tile([1, NB], F32, tag="rs", name=f"rs{pr}{lh}{c}")
                        for c in range(2)
                    ]

                    # phase A: scores -> exp -> rowsum accumulation
                    p_tiles = []
                    for i in range(NT):
                        p_i = ppool.tile([128, LHW], BF16, tag="p",
                                         name=f"p{pr}{lh}{i}")
                        p_tiles.append(p_i)
                        st = st_ps.tile([128, LHW], F32, tag="st",
                                        name=f"st{pr}{lh}{i}")
                        for c in range(2):
                            nc.tensor.matmul(
                                st[:, c * NB:(c + 1) * NB],
                                kt[:, i * 128:(i + 1) * 128],
                                qt[:, lh * LHW + c * NB: lh * LHW + (c + 1) * NB],
                                start=True,
                                stop=True,
                            )
                        nc.scalar.activation(
                            p_i[:],
                            st[:],
                            mybir.ActivationFunctionType.Exp,
                            scale=SCALE,
                        )
                        for c in range(2):
                            nc.tensor.matmul(
                                rs[c][:],
                                ones[:],
                                p_i[:, c * NB:(c + 1) * NB],
                                start=(i == 0),
                                stop=(i == NT - 1),
                            )

                    # reciprocal + broadcast overlap with the PV pass below
                    recip = tailpool.tile([1, LHW], F32, tag="recip")
                    for c in range(2):
                        nc.vector.reciprocal_approx_fast(
                            out=recip[:, c * NB:(c + 1) * NB], in_=rs[c][:]
                        )
                    rrep = tailpool.tile([128, LHW], F32, tag="rrep")
                    nc.gpsimd.partition_broadcast(rrep[:], recip[:])

                    # phase B: PV accumulation
                    for i in range(NT):
                        for c in range(2):
                            nc.tensor.matmul(
                                pv[c][:],
                                vt[:, i, :],
                                p_tiles[i][:, c * NB:(c + 1) * NB],
                                start=(i == 0),
                                stop=(i == NT - 1),
                            )

                    o_sb = opool.tile([128, LHW], F32, tag="o")
                    for c in range(2):
                        nc.vector.tensor_mul(
                            o_sb[:, c * NB:(c + 1) * NB],
                            pv[c][:],
                            rrep[:, c * NB:(c + 1) * NB],
                        )
                        nc.sync.dma_start(
                            out=out[pr, :, lh * LHW + c * NB: lh * LHW + (c + 1) * NB],
                            in_=o_sb[:, c * NB:(c + 1) * NB],
                        )

    nc.compile()
    return nc


def _get_nc():
    if "nc" not in _CACHE:
        _CACHE["nc"] = _build()
    return _CACHE["nc"]


def _prep_inputs(q_val, q_pos, k_val, k_pos, v_val, v_pos):
    bf16 = ml_dtypes.bfloat16
    # [B,L,H,2E] -> [B,H,2E,L] -> [16, 128, L]
    qcat = np.concatenate([q_val, q_pos], axis=-1).transpose(0, 2, 3, 1)
    qcatT = np.ascontiguousarray(qcat.reshape(B * H, 2 * E, L)).astype(bf16)
    kcat = np.concatenate([k_val, k_pos], axis=-1).transpose(0, 2, 3, 1)
    kcatT = np.ascontiguousarray(kcat.reshape(B * H, 2 * E, S)).astype(bf16)
    # [B,S,H,2E] -> [B,H,S,2E] -> [16, NT, 128, 128] -> [16, 128(s), NT, 128(e)]
    vcat = np.concatenate([v_val, v_pos], axis=-1).transpose(0, 2, 1, 3)
    vtiles = vcat.reshape(B * H, NT, 128, 2 * E).transpose(0, 2, 1, 3)
    vtiles = np.ascontiguousarray(vtiles).astype(bf16)
    return [
        {
            "qT": np.ascontiguousarray(qcatT[2 * c: 2 * c + 2]),
            "kT": np.ascontiguousarray(kcatT[2 * c: 2 * c + 2]),
            "vc": np.ascontiguousarray(vtiles[2 * c: 2 * c + 2]),
        }
        for c in range(NCORES)
    ]


def kernel(q_val, q_pos, k_val, k_pos, v_val, v_pos, attn_mask=None):
    q_val, q_pos, k_val, k_pos, v_val, v_pos = (
        np.asarray(x, dtype=np.float32)
        for x in (q_val, q_pos, k_val, k_pos, v_val, v_pos)
    )
    nc = _get_nc()
    in_maps = _prep_inputs(q_val, q_pos, k_val, k_pos, v_val, v_pos)
    res = run_bass_kernel_spmd(nc, in_maps, core_ids=list(range(NCORES)))
    _CACHE["last_results"] = res
    outs = np.stack([np.asarray(r["out"], dtype=np.float32) for r in res.results])
    # [8, NPAIR, 128, L] -> [16, 128, L] -> [B, H, L, 128]
    o = outs.reshape(B * H, 2 * E, L).transpose(0, 2, 1).reshape(B, H, L, 2 * E)
    V = np.ascontiguousarray(o[..., :E].transpose(0, 2, 1, 3), dtype=np.float32)
    Vp = np.ascontiguousarray(o[..., E:].transpose(0, 2, 1, 3), dtype=np.float32)
    return (V, Vp, None)
